# revision 12
# baseline (speedup 1.0000x reference)
"""DRNN encoder (3 dilated GRU layers) as a Bass/Tile kernel on 8 NeuronCores.

Data-parallel over the batch (4096 sentences -> 512/core). On-chip layout keeps
activations transposed: [H=128 partitions, (t-major: t*B + b) free]. With the
t-major ordering, layer l's dilated "step" (rate 2^l) is a contiguous
[128, rate*512] column block, so the whole dilation structure costs nothing.

v2 changes vs the gather-based baseline:
  - Embedding lookup runs on the HOST (numpy fancy-index); the kernel DMAs a
    precomputed transposed fp16 activation block [128, T*512] straight into
    SBUF (chunked so layer 0 starts early). Kills the 240us software-DGE
    gather that paced layer 0.
  - Output is written back as flat fp16 [128, T*512]; the host transposes to
    [B, T, H] f32 and applies the sentence mask. Kills the on-chip PE
    transposes + f32 casts + mask multiplies and halves output DMA bytes.
  - No identity-matmul: n's pre-activation is a scalar_tensor_tensor.
  - Elementwise work is split across DVE and the (otherwise idle) GpSimd/Pool
    engine for layers 1-2.
  - Layers are emission-interleaved (L1 step s after L0 step 2s+1, L2 step p
    after L1 step 2p+1) so L1/L2 throughput work fills L0's serial-chain
    stalls.

Per GRU chunk (<=512 cols): 6 PE matmuls (ih/hh x r/z/n, PSUM-accumulated),
2 sigmoids + 1 tanh on ACT, and tm/npre/d/u/h' elementwise on DVE/Pool.
"""
import sys

sys.path.insert(0, "/opt/trn_rl_repo")

import numpy as np

B, T, H, LAYERS = 4096, 50, 128, 3
NCORES = 8
BC = B // NCORES          # 512 sentences per core
CH = 512                  # column chunk (1 PSUM bank at f32)
TOK = T * BC              # 25600 tokens per core, t-major
T2 = ((T + 3) // 4) * 4   # layer-2 padded T (rate 4)
TOK2 = T2 * BC
NDMA_IN = 10              # input DMA chunks (5 timesteps each)

_CACHE = {}


def _build(l0_streams=2, cfg1=None, cfg2=None, cfg0=None, fill0=0, fill12=0):
    import concourse.bass as bass
    import concourse.bacc as bacc
    import concourse.tile as tile
    import concourse.mybir as mybir

    # engine placement per layer: npre in {'dve','idmm'} (GPSIMD cannot read
    # PSUM), d/u/hp in {'dve','pool'}
    cfg0 = cfg0 or dict(npre='idmm', d='dve', u='dve', hp='dve')
    cfg1 = cfg1 or dict(npre='idmm', d='pool', u='dve', hp='dve')
    cfg2 = cfg2 or dict(npre='idmm', d='pool', u='dve', hp='dve')

    FP16 = mybir.dt.float16
    F32 = mybir.dt.float32
    SIG = mybir.ActivationFunctionType.Sigmoid
    TANH = mybir.ActivationFunctionType.Tanh
    ADD = mybir.AluOpType.add
    MULT = mybir.AluOpType.mult

    nc = bacc.Bacc("TRN2", target_bir_lowering=False, debug=False)

    xt_d = nc.declare_dram_parameter("xt", [128, TOK], FP16, isOutput=False)
    wts = nc.declare_dram_parameter("wts", [128, LAYERS * 2 * 3 * H], FP16, isOutput=False)
    bias = nc.declare_dram_parameter("bias", [128, LAYERS * 4], F32, isOutput=False)
    out_d = nc.declare_dram_parameter("out", [128, TOK], FP16, isOutput=True)

    def eng(nm):
        return nc.gpsimd if nm == 'pool' else nc.vector

    with tile.TileContext(nc) as tc:
        with (
            tc.tile_pool(name="const", bufs=1) as const,
            tc.tile_pool(name="big", bufs=1) as big,
            tc.tile_pool(name="small", bufs=3) as small,
            tc.tile_pool(name="stage", bufs=2) as stage_p,
            tc.tile_pool(name="ps", bufs=2, space="PSUM") as ps,
        ):
            w_sb = const.tile([128, LAYERS * 2 * 3 * H], FP16)
            nc.sync.dma_start(out=w_sb[:], in_=wts[:])
            b_sb = const.tile([128, LAYERS * 4], F32)
            nc.sync.dma_start(out=b_sb[:], in_=bias[:])

            xt = big.tile([128, 1, TOK], FP16, tag="x0")
            x1 = big.tile([128, 1, TOK], FP16, tag="x1")
            x2 = big.tile([128, 1, TOK2], FP16, tag="x2")

            gch = TOK // NDMA_IN
            for c in range(NDMA_IN):
                nc.sync.dma_start(out=xt[:, :, c * gch:(c + 1) * gch],
                                  in_=xt_d[:, c * gch:(c + 1) * gch])
            nc.vector.memset(x2[:, :, TOK:TOK2], 0.0)  # layer-2 input padding

            def wt(l, io, g):
                o = (l * 2 * 3 + io * 3 + g) * H
                return w_sb[:, o:o + H]

            def bap(l, k):
                return b_sb[:, l * 4 + k:l * 4 + k + 1]

            def pe_fill(k):
                # Throwaway weight loads: dependency-free PE work that keeps
                # the tensor engine busy through pipeline waits, so it never
                # drops out of its ramped p-state (idle gaps halve the PE
                # clock for the next ~3us of work).
                for _ in range(k):
                    nc.tensor.ldweights(w_sb[:, 0:H])

            def emit_head(l, x, h, hp, first, cfg, fill=0):
                """Matmuls + sigmoids + tm + n-tanh for one <=512-col cell
                update. Returns state for emit_tail (the h'-assembly), which
                may be emitted later so other chunks' work can fill engine
                queues in between."""
                chk = x.shape[-1]
                rps = ps.tile([128, CH], F32, tag="rp", name="rps")[:, :chk]
                zps = ps.tile([128, CH], F32, tag="zp", name="zps")[:, :chk]
                nis = ps.tile([128, CH], F32, tag="ni", name="nis")[:, :chk]
                use_idmm = cfg['npre'] == 'idmm'
                nc.tensor.matmul(rps, wt(l, 0, 0), x, start=True, stop=first)
                nc.tensor.matmul(zps, wt(l, 0, 1), x, start=True, stop=first)
                nc.tensor.matmul(nis, wt(l, 0, 2), x, start=True,
                                 stop=not use_idmm)
                if not first:
                    nhs = ps.tile([128, CH], F32, tag="nh", name="nhs")[:, :chk]
                    nc.tensor.matmul(rps, wt(l, 1, 0), h, start=False, stop=True)
                    nc.tensor.matmul(zps, wt(l, 1, 1), h, start=False, stop=True)
                    nc.tensor.matmul(nhs, wt(l, 1, 2), h, start=True, stop=True)
                r = small.tile([128, CH], FP16, name="r", tag="r", bufs=4)[:, :chk]
                z = small.tile([128, CH], FP16, name="z", tag="z", bufs=8)[:, :chk]
                nc.scalar.activation(r, rps, SIG, bias=bap(l, 0))
                nc.scalar.activation(z, zps, SIG, bias=bap(l, 1))
                tm = small.tile([128, CH], FP16, name="tm", tag="tm", bufs=6)[:, :chk]
                if first:
                    nc.vector.tensor_scalar_mul(tm, r, bap(l, 3))
                else:
                    nc.vector.scalar_tensor_tensor(tm, nhs, bap(l, 3), r,
                                                   op0=ADD, op1=MULT)
                pe_fill(fill)
                if use_idmm:
                    nc.tensor.matmul(nis, ident, tm, start=False, stop=True)
                    n_src, n_bias = nis, bap(l, 2)
                else:
                    npre = small.tile([128, CH], FP16, name="npre", tag="np",
                                      bufs=4)[:, :chk]
                    eng(cfg['npre']).scalar_tensor_tensor(
                        npre, nis, bap(l, 2), tm, op0=ADD, op1=ADD)
                    n_src, n_bias = npre, 0.0
                n = small.tile([128, CH], FP16, name="n", tag="n", bufs=6)[:, :chk]
                nc.scalar.activation(n, n_src, TANH, bias=n_bias)
                return (h, hp, z, n, first, cfg, chk)

            def emit_tail(state):
                h, hp, z, n, first, cfg, chk = state
                if first:
                    e = small.tile([128, CH], FP16, name="e", tag="d",
                                   bufs=4)[:, :chk]
                    nc.vector.tensor_mul(e, z, n)
                    nc.vector.tensor_sub(hp, n, e)
                else:
                    d = small.tile([128, CH], FP16, name="d", tag="d",
                                   bufs=4)[:, :chk]
                    eng(cfg['d']).tensor_sub(d, h, n)
                    u = small.tile([128, CH], FP16, name="u", tag="u",
                                   bufs=4)[:, :chk]
                    eng(cfg['u']).tensor_mul(u, z, d)
                    eng(cfg['hp']).tensor_add(hp, n, u)

            def emit_chunk(l, x, h, hp, first, cfg, fill=0):
                emit_tail(emit_head(l, x, h, hp, first, cfg, fill))

            ident = None
            if 'idmm' in (cfg0['npre'], cfg1['npre'], cfg2['npre']):
                from concourse.masks import make_identity
                ident = const.tile([H, H], FP16)
                make_identity(nc, ident)

            def emit_l0_heads(t):
                nch = CH // l0_streams
                states = []
                for s in range(l0_streams):
                    c0 = t * CH + s * nch
                    x = xt[:, 0, c0:c0 + nch]
                    h = x1[:, 0, c0 - CH:c0 - CH + nch] if t > 0 else None
                    hp = x1[:, 0, c0:c0 + nch]
                    states.append(emit_head(0, x, h, hp, t == 0, cfg0,
                                            fill=fill0))
                return states

            def emit_l1_chunk(s, k):
                c0 = s * 1024 + k * CH
                x = x1[:, 0, c0:c0 + CH]
                h = x2[:, 0, c0 - 1024:c0 - 1024 + CH] if s > 0 else None
                hp = x2[:, 0, c0:c0 + CH]
                emit_chunk(1, x, h, hp, s == 0, cfg1, fill=fill12)

            def emit_l1(s):
                for k in range(2):
                    emit_l1_chunk(s, k)

            stage_prev = [None, None]  # [previous step's tile, current]

            def emit_l2(p, half):
                # half 0: chunks 0,1 (allocates this step's stage tile);
                # half 1: chunks 2,3 + output DMA. Split so every odd
                # window carries ready L2 matmuls ahead of L0's stall.
                if half == 0:
                    stage_prev[1] = stage_p.tile([128, 2048], FP16, tag="st",
                                                 name="st")
                st = stage_prev[1]
                for k in (0, 1) if half == 0 else (2, 3):
                    c0 = p * 2048 + k * CH
                    x = x2[:, 0, c0:c0 + CH]
                    h = (stage_prev[0][:, k * CH:(k + 1) * CH]
                         if p > 0 else None)
                    hp = st[:, k * CH:(k + 1) * CH]
                    emit_chunk(2, x, h, hp, p == 0, cfg2, fill=fill12)
                if half == 1:
                    w = min(TOK - p * 2048, 2048)
                    nc.sync.dma_start(out=out_d[:, p * 2048:p * 2048 + w],
                                      in_=st[:, :w])
                    stage_prev[0] = st

            # Emission order note: each engine executes its stream IN ORDER,
            # so ready work must be emitted BEFORE stall-prone work. Per
            # window t we first emit the L1/L2 steps whose inputs completed
            # in earlier windows, then L0 step t (whose hh-matmuls stall on
            # h'(t-1)); the ready L1/L2 matmuls keep the PE busy through
            # L0's serial-chain hole.
            # Window weave: L1's first chunk covers engine queues while
            # L0's matmuls wait on h'(t-1); L0's chain ops (sigmoids, tm,
            # idmm, tanh) go next so they never sit behind a full window of
            # queued L1/L2 work; remaining L1/L2 chunks refill the queues;
            # L0's h'-assembly is emitted last, completing just before the
            # next window's L0 matmuls need it.
            for t in range(T):
                l1s = (t - 2) // 2 if (t >= 2 and t % 2 == 0) else None
                if l1s is not None:
                    emit_l1_chunk(l1s, 0)
                l0_states = emit_l0_heads(t)
                if l1s is not None:
                    emit_l1_chunk(l1s, 1)
                if t >= 5 and t % 4 == 1:
                    emit_l2((t - 5) // 4, 0)
                if t >= 7 and t % 4 == 3:
                    emit_l2((t - 7) // 4, 1)
                for st_ in l0_states:
                    emit_tail(st_)
            emit_l1(24)
            emit_l2(11, 1)
            emit_l2(12, 0)
            emit_l2(12, 1)

    nc.finalize()
    return nc


def _get_runner():
    if "runner" in _CACHE:
        return _CACHE["runner"]
    import jax
    import numpy as _np
    from jax.sharding import Mesh, PartitionSpec
    from jax.experimental.shard_map import shard_map
    import concourse.bass2jax as bass2jax
    import concourse.mybir as mybir

    nc = _CACHE.get("nc") or _build()
    _CACHE["nc"] = nc
    bass2jax.install_neuronx_cc_hook()

    partition_name = nc.partition_id_tensor.name if nc.partition_id_tensor else None
    in_names, out_names, out_avals, zero_outs = [], [], [], []
    for alloc in nc.m.functions[0].allocations:
        if not isinstance(alloc, mybir.MemoryLocationSet):
            continue
        name = alloc.memorylocations[0].name
        if alloc.kind == "ExternalInput":
            if name != partition_name:
                in_names.append(name)
        elif alloc.kind == "ExternalOutput":
            out_avals.append(jax.core.ShapedArray(
                tuple(alloc.tensor_shape), mybir.dt.np(alloc.dtype)))
            zero_outs.append(_np.zeros(alloc.tensor_shape, mybir.dt.np(alloc.dtype)))
            out_names.append(name)

    n_params = len(in_names)
    all_in_names = list(in_names) + list(out_names)
    if partition_name is not None:
        all_in_names.append(partition_name)

    donate = tuple(range(n_params, n_params + len(out_names)))

    def _body(*args):
        operands = list(args)
        if partition_name is not None:
            operands.append(bass2jax.partition_id_tensor())
        outs = bass2jax._bass_exec_p.bind(
            *operands,
            out_avals=tuple(out_avals),
            in_names=tuple(all_in_names),
            out_names=tuple(out_names),
            lowering_input_output_aliases=(),
            sim_require_finite=True,
            sim_require_nnan=True,
            nc=nc,
        )
        return tuple(outs)

    devices = jax.devices()[:NCORES]
    mesh = Mesh(_np.asarray(devices), ("core",))
    in_specs = (PartitionSpec("core"),) * (n_params + len(out_names))
    out_specs = (PartitionSpec("core"),) * len(out_names)
    sharded = jax.jit(
        shard_map(_body, mesh=mesh, in_specs=in_specs, out_specs=out_specs,
                  check_rep=False),
        donate_argnums=donate, keep_unused=True)

    def run(in_maps):
        concat_in = [
            _np.concatenate([_np.asarray(m[name]) for m in in_maps], axis=0)
            for name in in_names
        ]
        concat_zeros = [
            _np.zeros((NCORES * z.shape[0], *z.shape[1:]), z.dtype)
            for z in zero_outs
        ]
        out_arrs = sharded(*concat_in, *concat_zeros)
        o = _np.asarray(out_arrs[out_names.index("out")])
        return o.reshape(NCORES, 128, TOK)

    _CACHE["runner"] = run
    return run


def make_core_inputs(text_sh, emb16, w_np, b_np):
    """Per-core input dict: host embedding lookup in transposed t-major fp16."""
    x = emb16[text_sh]                                   # [BC, T, H] fp16
    xt = np.ascontiguousarray(x.transpose(2, 1, 0)).reshape(128, TOK)
    return {"xt": xt, "wts": w_np, "bias": b_np}


def pack_weights(params):
    w_np = np.zeros((128, LAYERS * 2 * 3 * H), np.float16)
    b_np = np.zeros((128, LAYERS * 4), np.float32)
    for l, (Wih, Whh, bih, bhh) in enumerate(params):
        for g in range(3):
            w_np[:, (l * 6 + g) * H:(l * 6 + g + 1) * H] = \
                Wih[g * H:(g + 1) * H, :].T.astype(np.float16)
            w_np[:, (l * 6 + 3 + g) * H:(l * 6 + 3 + g + 1) * H] = \
                Whh[g * H:(g + 1) * H, :].T.astype(np.float16)
        b_np[:, l * 4 + 0] = bih[0:H] + bhh[0:H]
        b_np[:, l * 4 + 1] = bih[H:2 * H] + bhh[H:2 * H]
        b_np[:, l * 4 + 2] = bih[2 * H:3 * H]
        b_np[:, l * 4 + 3] = bhh[2 * H:3 * H]
    return w_np, b_np


def prepare_in_maps(text_inputs, emb, params):
    emb16 = np.asarray(emb, np.float32).astype(np.float16)
    w_np, b_np = pack_weights(params)
    return [
        make_core_inputs(np.asarray(text_inputs)[c * BC:(c + 1) * BC],
                         emb16, w_np, b_np)
        for c in range(NCORES)
    ]


def finish_output(o, text_inputs):
    """[NCORES,128,TOK] fp16 -> [B,T,H] f32 with empty-sentence masking."""
    o = o.reshape(NCORES, 128, T, BC).transpose(0, 3, 2, 1)  # [c, b, t, h]
    out = np.ascontiguousarray(o).reshape(B, T, H).astype(np.float32)
    lens = np.sign(np.asarray(text_inputs)).sum(axis=1)
    out *= (lens > 0).astype(np.float32)[:, None, None]
    return out


def kernel(text_inputs, mask_input, len_seq, emb,
           Wih0, Whh0, bih0, bhh0, Wih1, Whh1, bih1, bhh1,
           Wih2, Whh2, bih2, bhh2):
    run = _get_runner()
    params = [(np.asarray(Wih0, np.float32), np.asarray(Whh0, np.float32),
               np.asarray(bih0, np.float32), np.asarray(bhh0, np.float32)),
              (np.asarray(Wih1, np.float32), np.asarray(Whh1, np.float32),
               np.asarray(bih1, np.float32), np.asarray(bhh1, np.float32)),
              (np.asarray(Wih2, np.float32), np.asarray(Whh2, np.float32),
               np.asarray(bih2, np.float32), np.asarray(bhh2, np.float32))]
    in_maps = prepare_in_maps(text_inputs, emb, params)
    o = run(in_maps)
    return finish_output(o, text_inputs)


# revision 13
# speedup vs baseline: 1.4757x; 1.4757x over previous
"""DRNN encoder (3 dilated GRU layers) as a Bass/Tile kernel on 8 NeuronCores.

Data-parallel over the batch (4096 sentences -> 512/core). On-chip layout keeps
activations transposed: [H=128 partitions, (t-major: t*B + b) free]. With the
t-major ordering, layer l's dilated "step" (rate 2^l) is a contiguous
[128, rate*512] column block, so the whole dilation structure costs nothing.

v2 changes vs the gather-based baseline:
  - Embedding lookup runs on the HOST (numpy fancy-index); the kernel DMAs a
    precomputed transposed fp16 activation block [128, T*512] straight into
    SBUF (chunked so layer 0 starts early). Kills the 240us software-DGE
    gather that paced layer 0.
  - Output is written back as flat fp16 [128, T*512]; the host transposes to
    [B, T, H] f32 and applies the sentence mask. Kills the on-chip PE
    transposes + f32 casts + mask multiplies and halves output DMA bytes.
  - No identity-matmul: n's pre-activation is a scalar_tensor_tensor.
  - Elementwise work is split across DVE and the (otherwise idle) GpSimd/Pool
    engine for layers 1-2.
  - Layers are emission-interleaved (L1 step s after L0 step 2s+1, L2 step p
    after L1 step 2p+1) so L1/L2 throughput work fills L0's serial-chain
    stalls.

Per GRU chunk (<=512 cols): 6 PE matmuls (ih/hh x r/z/n, PSUM-accumulated),
2 sigmoids + 1 tanh on ACT, and tm/npre/d/u/h' elementwise on DVE/Pool.
"""
import sys

sys.path.insert(0, "/opt/trn_rl_repo")

import numpy as np

B, T, H, LAYERS = 4096, 50, 128, 3
NCORES = 8
BC = B // NCORES          # 512 sentences per core
CH = 512                  # column chunk (1 PSUM bank at f32)
TOK = T * BC              # 25600 tokens per core, t-major
T2 = ((T + 3) // 4) * 4   # layer-2 padded T (rate 4)
TOK2 = T2 * BC
NDMA_IN = 10              # input DMA chunks (5 timesteps each)

_CACHE = {}


def _build(l0_streams=2, cfg1=None, cfg2=None, cfg0=None, fill0=0, fill12=0):
    import concourse.bass as bass
    import concourse.bacc as bacc
    import concourse.tile as tile
    import concourse.mybir as mybir

    # engine placement per layer: npre in {'dve','idmm'} (GPSIMD cannot read
    # PSUM), d/u/hp in {'dve','pool'}
    cfg0 = cfg0 or dict(npre='idmm', d='dve', u='dve', hp='dve')
    cfg1 = cfg1 or dict(npre='idmm', d='pool', u='dve', hp='dve')
    cfg2 = cfg2 or dict(npre='idmm', d='pool', u='dve', hp='dve')

    FP16 = mybir.dt.float16
    F32 = mybir.dt.float32
    SIG = mybir.ActivationFunctionType.Sigmoid
    TANH = mybir.ActivationFunctionType.Tanh
    ADD = mybir.AluOpType.add
    MULT = mybir.AluOpType.mult

    nc = bacc.Bacc("TRN2", target_bir_lowering=False, debug=False)

    xt_d = nc.declare_dram_parameter("xt", [128, TOK], FP16, isOutput=False)
    wts = nc.declare_dram_parameter("wts", [128, LAYERS * 2 * 3 * H], FP16, isOutput=False)
    bias = nc.declare_dram_parameter("bias", [128, LAYERS * 4], F32, isOutput=False)
    out_d = nc.declare_dram_parameter("out", [128, TOK], FP16, isOutput=True)

    def eng(nm):
        return nc.gpsimd if nm == 'pool' else nc.vector

    with tile.TileContext(nc) as tc:
        with (
            tc.tile_pool(name="const", bufs=1) as const,
            tc.tile_pool(name="big", bufs=1) as big,
            tc.tile_pool(name="small", bufs=3) as small,
            tc.tile_pool(name="stage", bufs=2) as stage_p,
            tc.tile_pool(name="ps", bufs=2, space="PSUM") as ps,
        ):
            w_sb = const.tile([128, LAYERS * 2 * 3 * H], FP16)
            nc.sync.dma_start(out=w_sb[:], in_=wts[:])
            b_sb = const.tile([128, LAYERS * 4], F32)
            nc.sync.dma_start(out=b_sb[:], in_=bias[:])

            xt = big.tile([128, 1, TOK], FP16, tag="x0")
            x1 = big.tile([128, 1, TOK], FP16, tag="x1")
            x2 = big.tile([128, 1, TOK2], FP16, tag="x2")

            gch = TOK // NDMA_IN
            for c in range(NDMA_IN):
                nc.sync.dma_start(out=xt[:, :, c * gch:(c + 1) * gch],
                                  in_=xt_d[:, c * gch:(c + 1) * gch])
            nc.vector.memset(x2[:, :, TOK:TOK2], 0.0)  # layer-2 input padding

            def wt(l, io, g):
                o = (l * 2 * 3 + io * 3 + g) * H
                return w_sb[:, o:o + H]

            def bap(l, k):
                return b_sb[:, l * 4 + k:l * 4 + k + 1]

            def pe_fill(k):
                # Throwaway weight loads: dependency-free PE work that keeps
                # the tensor engine busy through pipeline waits, so it never
                # drops out of its ramped p-state (idle gaps halve the PE
                # clock for the next ~3us of work).
                for _ in range(k):
                    nc.tensor.ldweights(w_sb[:, 0:H])

            def emit_head(l, x, h, hp, first, cfg, fill=0):
                """Matmuls + sigmoids + tm + n-tanh for one <=512-col cell
                update. Returns state for emit_tail (the h'-assembly), which
                may be emitted later so other chunks' work can fill engine
                queues in between."""
                chk = x.shape[-1]
                rps = ps.tile([128, CH], F32, tag="rp", name="rps")[:, :chk]
                zps = ps.tile([128, CH], F32, tag="zp", name="zps")[:, :chk]
                nis = ps.tile([128, CH], F32, tag="ni", name="nis")[:, :chk]
                use_idmm = cfg['npre'] == 'idmm'
                nc.tensor.matmul(rps, wt(l, 0, 0), x, start=True, stop=first)
                nc.tensor.matmul(zps, wt(l, 0, 1), x, start=True, stop=first)
                nc.tensor.matmul(nis, wt(l, 0, 2), x, start=True,
                                 stop=not use_idmm)
                if not first:
                    nhs = ps.tile([128, CH], F32, tag="nh", name="nhs")[:, :chk]
                    nc.tensor.matmul(rps, wt(l, 1, 0), h, start=False, stop=True)
                    nc.tensor.matmul(zps, wt(l, 1, 1), h, start=False, stop=True)
                    nc.tensor.matmul(nhs, wt(l, 1, 2), h, start=True, stop=True)
                r = small.tile([128, CH], FP16, name="r", tag="r", bufs=4)[:, :chk]
                z = small.tile([128, CH], FP16, name="z", tag="z", bufs=8)[:, :chk]
                nc.scalar.activation(r, rps, SIG, bias=bap(l, 0))
                nc.scalar.activation(z, zps, SIG, bias=bap(l, 1))
                tm = small.tile([128, CH], FP16, name="tm", tag="tm", bufs=6)[:, :chk]
                if first:
                    nc.vector.tensor_scalar_mul(tm, r, bap(l, 3))
                else:
                    nc.vector.scalar_tensor_tensor(tm, nhs, bap(l, 3), r,
                                                   op0=ADD, op1=MULT)
                pe_fill(fill)
                if use_idmm:
                    nc.tensor.matmul(nis, ident, tm, start=False, stop=True)
                    n_src, n_bias = nis, bap(l, 2)
                else:
                    npre = small.tile([128, CH], FP16, name="npre", tag="np",
                                      bufs=4)[:, :chk]
                    eng(cfg['npre']).scalar_tensor_tensor(
                        npre, nis, bap(l, 2), tm, op0=ADD, op1=ADD)
                    n_src, n_bias = npre, 0.0
                n = small.tile([128, CH], FP16, name="n", tag="n", bufs=6)[:, :chk]
                nc.scalar.activation(n, n_src, TANH, bias=n_bias)
                return (h, hp, z, n, first, cfg, chk)

            def emit_tail(state):
                h, hp, z, n, first, cfg, chk = state
                if first:
                    e = small.tile([128, CH], FP16, name="e", tag="d",
                                   bufs=4)[:, :chk]
                    nc.vector.tensor_mul(e, z, n)
                    nc.vector.tensor_sub(hp, n, e)
                else:
                    d = small.tile([128, CH], FP16, name="d", tag="d",
                                   bufs=4)[:, :chk]
                    eng(cfg['d']).tensor_sub(d, h, n)
                    u = small.tile([128, CH], FP16, name="u", tag="u",
                                   bufs=4)[:, :chk]
                    eng(cfg['u']).tensor_mul(u, z, d)
                    eng(cfg['hp']).tensor_add(hp, n, u)

            def emit_chunk(l, x, h, hp, first, cfg, fill=0):
                emit_tail(emit_head(l, x, h, hp, first, cfg, fill))

            ident = None
            if 'idmm' in (cfg0['npre'], cfg1['npre'], cfg2['npre']):
                from concourse.masks import make_identity
                ident = const.tile([H, H], FP16)
                make_identity(nc, ident)

            def emit_l0_heads(t):
                nch = CH // l0_streams
                states = []
                for s in range(l0_streams):
                    c0 = t * CH + s * nch
                    x = xt[:, 0, c0:c0 + nch]
                    h = x1[:, 0, c0 - CH:c0 - CH + nch] if t > 0 else None
                    hp = x1[:, 0, c0:c0 + nch]
                    states.append(emit_head(0, x, h, hp, t == 0, cfg0,
                                            fill=fill0))
                return states

            def emit_l1_chunk(s, k):
                c0 = s * 1024 + k * CH
                x = x1[:, 0, c0:c0 + CH]
                h = x2[:, 0, c0 - 1024:c0 - 1024 + CH] if s > 0 else None
                hp = x2[:, 0, c0:c0 + CH]
                emit_chunk(1, x, h, hp, s == 0, cfg1, fill=fill12)

            def emit_l1(s):
                for k in range(2):
                    emit_l1_chunk(s, k)

            stage_prev = [None, None]  # [previous step's tile, current]

            def emit_l2_chunk(p, k):
                # k==0 allocates step p's stage tile; k==3 appends the
                # output DMA and retires the step.
                if k == 0:
                    stage_prev[1] = stage_p.tile([128, 2048], FP16, tag="st",
                                                 name="st")
                st = stage_prev[1]
                c0 = p * 2048 + k * CH
                x = x2[:, 0, c0:c0 + CH]
                h = (stage_prev[0][:, k * CH:(k + 1) * CH]
                     if p > 0 else None)
                hp = st[:, k * CH:(k + 1) * CH]
                emit_chunk(2, x, h, hp, p == 0, cfg2, fill=fill12)
                if k == 3:
                    w = min(TOK - p * 2048, 2048)
                    nc.sync.dma_start(out=out_d[:, p * 2048:p * 2048 + w],
                                      in_=st[:, :w])
                    stage_prev[0] = st

            # Emission order note: each engine executes its stream IN ORDER,
            # so ready work must be emitted BEFORE stall-prone work. Per
            # window t we first emit the L1/L2 steps whose inputs completed
            # in earlier windows, then L0 step t (whose hh-matmuls stall on
            # h'(t-1)); the ready L1/L2 matmuls keep the PE busy through
            # L0's serial-chain hole.
            # Sandwich weave: [cover chunk, L0 streams, cover chunk] per
            # window. The leading cover chunk keeps engines busy while
            # L0(t)'s matmuls wait on h'(t-1); emitting L0 in the middle
            # puts its serial-chain ops near the front of every engine
            # queue, so h'(t) completes mid-window; the trailing cover
            # chunk refills the queues behind it. Even windows carry L1
            # chunks as cover, odd windows L2 chunks.
            def covers(t):
                if t % 2 == 0:
                    s = (t - 2) // 2
                    return [] if t < 2 else [('l1', s, 0), ('l1', s, 1)]
                if t % 4 == 1:
                    p = (t - 5) // 4
                    return [] if t < 5 else [('l2', p, 0), ('l2', p, 1)]
                p = (t - 7) // 4
                return [] if t < 7 else [('l2', p, 2), ('l2', p, 3)]

            def emit_cover(c):
                if c[0] == 'l1':
                    emit_l1_chunk(c[1], c[2])
                else:
                    emit_l2_chunk(c[1], c[2])

            for t in range(T):
                cs = covers(t)
                if cs:
                    emit_cover(cs[0])
                for st_ in emit_l0_heads(t):
                    emit_tail(st_)
                if cs:
                    emit_cover(cs[1])
            emit_l1(24)
            emit_l2_chunk(11, 2)
            emit_l2_chunk(11, 3)
            for k in range(4):
                emit_l2_chunk(12, k)

    nc.finalize()
    return nc


def _get_runner():
    if "runner" in _CACHE:
        return _CACHE["runner"]
    import jax
    import numpy as _np
    from jax.sharding import Mesh, PartitionSpec
    from jax.experimental.shard_map import shard_map
    import concourse.bass2jax as bass2jax
    import concourse.mybir as mybir

    nc = _CACHE.get("nc") or _build()
    _CACHE["nc"] = nc
    bass2jax.install_neuronx_cc_hook()

    partition_name = nc.partition_id_tensor.name if nc.partition_id_tensor else None
    in_names, out_names, out_avals, zero_outs = [], [], [], []
    for alloc in nc.m.functions[0].allocations:
        if not isinstance(alloc, mybir.MemoryLocationSet):
            continue
        name = alloc.memorylocations[0].name
        if alloc.kind == "ExternalInput":
            if name != partition_name:
                in_names.append(name)
        elif alloc.kind == "ExternalOutput":
            out_avals.append(jax.core.ShapedArray(
                tuple(alloc.tensor_shape), mybir.dt.np(alloc.dtype)))
            zero_outs.append(_np.zeros(alloc.tensor_shape, mybir.dt.np(alloc.dtype)))
            out_names.append(name)

    n_params = len(in_names)
    all_in_names = list(in_names) + list(out_names)
    if partition_name is not None:
        all_in_names.append(partition_name)

    donate = tuple(range(n_params, n_params + len(out_names)))

    def _body(*args):
        operands = list(args)
        if partition_name is not None:
            operands.append(bass2jax.partition_id_tensor())
        outs = bass2jax._bass_exec_p.bind(
            *operands,
            out_avals=tuple(out_avals),
            in_names=tuple(all_in_names),
            out_names=tuple(out_names),
            lowering_input_output_aliases=(),
            sim_require_finite=True,
            sim_require_nnan=True,
            nc=nc,
        )
        return tuple(outs)

    devices = jax.devices()[:NCORES]
    mesh = Mesh(_np.asarray(devices), ("core",))
    in_specs = (PartitionSpec("core"),) * (n_params + len(out_names))
    out_specs = (PartitionSpec("core"),) * len(out_names)
    sharded = jax.jit(
        shard_map(_body, mesh=mesh, in_specs=in_specs, out_specs=out_specs,
                  check_rep=False),
        donate_argnums=donate, keep_unused=True)

    def run(in_maps):
        concat_in = [
            _np.concatenate([_np.asarray(m[name]) for m in in_maps], axis=0)
            for name in in_names
        ]
        concat_zeros = [
            _np.zeros((NCORES * z.shape[0], *z.shape[1:]), z.dtype)
            for z in zero_outs
        ]
        out_arrs = sharded(*concat_in, *concat_zeros)
        o = _np.asarray(out_arrs[out_names.index("out")])
        return o.reshape(NCORES, 128, TOK)

    _CACHE["runner"] = run
    return run


def make_core_inputs(text_sh, emb16, w_np, b_np):
    """Per-core input dict: host embedding lookup in transposed t-major fp16."""
    x = emb16[text_sh]                                   # [BC, T, H] fp16
    xt = np.ascontiguousarray(x.transpose(2, 1, 0)).reshape(128, TOK)
    return {"xt": xt, "wts": w_np, "bias": b_np}


def pack_weights(params):
    w_np = np.zeros((128, LAYERS * 2 * 3 * H), np.float16)
    b_np = np.zeros((128, LAYERS * 4), np.float32)
    for l, (Wih, Whh, bih, bhh) in enumerate(params):
        for g in range(3):
            w_np[:, (l * 6 + g) * H:(l * 6 + g + 1) * H] = \
                Wih[g * H:(g + 1) * H, :].T.astype(np.float16)
            w_np[:, (l * 6 + 3 + g) * H:(l * 6 + 3 + g + 1) * H] = \
                Whh[g * H:(g + 1) * H, :].T.astype(np.float16)
        b_np[:, l * 4 + 0] = bih[0:H] + bhh[0:H]
        b_np[:, l * 4 + 1] = bih[H:2 * H] + bhh[H:2 * H]
        b_np[:, l * 4 + 2] = bih[2 * H:3 * H]
        b_np[:, l * 4 + 3] = bhh[2 * H:3 * H]
    return w_np, b_np


def prepare_in_maps(text_inputs, emb, params):
    emb16 = np.asarray(emb, np.float32).astype(np.float16)
    w_np, b_np = pack_weights(params)
    return [
        make_core_inputs(np.asarray(text_inputs)[c * BC:(c + 1) * BC],
                         emb16, w_np, b_np)
        for c in range(NCORES)
    ]


def finish_output(o, text_inputs):
    """[NCORES,128,TOK] fp16 -> [B,T,H] f32 with empty-sentence masking."""
    o = o.reshape(NCORES, 128, T, BC).transpose(0, 3, 2, 1)  # [c, b, t, h]
    out = np.ascontiguousarray(o).reshape(B, T, H).astype(np.float32)
    lens = np.sign(np.asarray(text_inputs)).sum(axis=1)
    out *= (lens > 0).astype(np.float32)[:, None, None]
    return out


def kernel(text_inputs, mask_input, len_seq, emb,
           Wih0, Whh0, bih0, bhh0, Wih1, Whh1, bih1, bhh1,
           Wih2, Whh2, bih2, bhh2):
    run = _get_runner()
    params = [(np.asarray(Wih0, np.float32), np.asarray(Whh0, np.float32),
               np.asarray(bih0, np.float32), np.asarray(bhh0, np.float32)),
              (np.asarray(Wih1, np.float32), np.asarray(Whh1, np.float32),
               np.asarray(bih1, np.float32), np.asarray(bhh1, np.float32)),
              (np.asarray(Wih2, np.float32), np.asarray(Whh2, np.float32),
               np.asarray(bih2, np.float32), np.asarray(bhh2, np.float32))]
    in_maps = prepare_in_maps(text_inputs, emb, params)
    o = run(in_maps)
    return finish_output(o, text_inputs)


# revision 14
# speedup vs baseline: 1.5625x; 1.0588x over previous
"""DRNN encoder (3 dilated GRU layers) as a Bass/Tile kernel on 8 NeuronCores.

Data-parallel over the batch (4096 sentences -> 512/core). On-chip layout keeps
activations transposed: [H=128 partitions, (t-major: t*B + b) free]. With the
t-major ordering, layer l's dilated "step" (rate 2^l) is a contiguous
[128, rate*512] column block, so the whole dilation structure costs nothing.

v2 changes vs the gather-based baseline:
  - Embedding lookup runs on the HOST (numpy fancy-index); the kernel DMAs a
    precomputed transposed fp16 activation block [128, T*512] straight into
    SBUF (chunked so layer 0 starts early). Kills the 240us software-DGE
    gather that paced layer 0.
  - Output is written back as flat fp16 [128, T*512]; the host transposes to
    [B, T, H] f32 and applies the sentence mask. Kills the on-chip PE
    transposes + f32 casts + mask multiplies and halves output DMA bytes.
  - No identity-matmul: n's pre-activation is a scalar_tensor_tensor.
  - Elementwise work is split across DVE and the (otherwise idle) GpSimd/Pool
    engine for layers 1-2.
  - Layers are emission-interleaved (L1 step s after L0 step 2s+1, L2 step p
    after L1 step 2p+1) so L1/L2 throughput work fills L0's serial-chain
    stalls.

Per GRU chunk (<=512 cols): 6 PE matmuls (ih/hh x r/z/n, PSUM-accumulated),
2 sigmoids + 1 tanh on ACT, and tm/npre/d/u/h' elementwise on DVE/Pool.
"""
import sys

sys.path.insert(0, "/opt/trn_rl_repo")

import numpy as np

B, T, H, LAYERS = 4096, 50, 128, 3
NCORES = 8
BC = B // NCORES          # 512 sentences per core
CH = 512                  # column chunk (1 PSUM bank at f32)
TOK = T * BC              # 25600 tokens per core, t-major
T2 = ((T + 3) // 4) * 4   # layer-2 padded T (rate 4)
TOK2 = T2 * BC
NDMA_IN = 10              # input DMA chunks (5 timesteps each)

_CACHE = {}


def _build(l0_streams=1, cfg1=None, cfg2=None, cfg0=None, fill0=0, fill12=0):
    import concourse.bass as bass
    import concourse.bacc as bacc
    import concourse.tile as tile
    import concourse.mybir as mybir

    # engine placement per layer: npre in {'dve','idmm'} (GPSIMD cannot read
    # PSUM), d/u/hp in {'dve','pool'}
    cfg0 = cfg0 or dict(npre='idmm', d='dve', u='dve', hp='dve')
    cfg1 = cfg1 or dict(npre='idmm', d='pool', u='dve', hp='dve')
    cfg2 = cfg2 or dict(npre='idmm', d='pool', u='dve', hp='dve')

    FP16 = mybir.dt.float16
    F32 = mybir.dt.float32
    SIG = mybir.ActivationFunctionType.Sigmoid
    TANH = mybir.ActivationFunctionType.Tanh
    ADD = mybir.AluOpType.add
    MULT = mybir.AluOpType.mult

    nc = bacc.Bacc("TRN2", target_bir_lowering=False, debug=False)

    xt_d = nc.declare_dram_parameter("xt", [128, TOK], FP16, isOutput=False)
    wts = nc.declare_dram_parameter("wts", [128, LAYERS * 2 * 3 * H], FP16, isOutput=False)
    bias = nc.declare_dram_parameter("bias", [128, LAYERS * 4], F32, isOutput=False)
    out_d = nc.declare_dram_parameter("out", [128, TOK], FP16, isOutput=True)

    def eng(nm):
        return nc.gpsimd if nm == 'pool' else nc.vector

    with tile.TileContext(nc) as tc:
        with (
            tc.tile_pool(name="const", bufs=1) as const,
            tc.tile_pool(name="big", bufs=1) as big,
            tc.tile_pool(name="small", bufs=3) as small,
            tc.tile_pool(name="stage", bufs=2) as stage_p,
            tc.tile_pool(name="ps", bufs=2, space="PSUM") as ps,
        ):
            w_sb = const.tile([128, LAYERS * 2 * 3 * H], FP16)
            nc.sync.dma_start(out=w_sb[:], in_=wts[:])
            b_sb = const.tile([128, LAYERS * 4], F32)
            nc.sync.dma_start(out=b_sb[:], in_=bias[:])

            xt = big.tile([128, 1, TOK], FP16, tag="x0")
            x1 = big.tile([128, 1, TOK], FP16, tag="x1")
            x2 = big.tile([128, 1, TOK2], FP16, tag="x2")

            gch = TOK // NDMA_IN
            for c in range(NDMA_IN):
                nc.sync.dma_start(out=xt[:, :, c * gch:(c + 1) * gch],
                                  in_=xt_d[:, c * gch:(c + 1) * gch])
            nc.vector.memset(x2[:, :, TOK:TOK2], 0.0)  # layer-2 input padding

            def wt(l, io, g):
                o = (l * 2 * 3 + io * 3 + g) * H
                return w_sb[:, o:o + H]

            def bap(l, k):
                return b_sb[:, l * 4 + k:l * 4 + k + 1]

            def pe_fill(k):
                # Throwaway weight loads: dependency-free PE work that keeps
                # the tensor engine busy through pipeline waits, so it never
                # drops out of its ramped p-state (idle gaps halve the PE
                # clock for the next ~3us of work).
                for _ in range(k):
                    nc.tensor.ldweights(w_sb[:, 0:H])

            def emit_head(l, x, h, hp, first, cfg, fill=0):
                """Matmuls + sigmoids + tm + n-tanh for one <=512-col cell
                update. Returns state for emit_tail (the h'-assembly), which
                may be emitted later so other chunks' work can fill engine
                queues in between."""
                chk = x.shape[-1]
                rps = ps.tile([128, CH], F32, tag="rp", name="rps")[:, :chk]
                zps = ps.tile([128, CH], F32, tag="zp", name="zps")[:, :chk]
                nis = ps.tile([128, CH], F32, tag="ni", name="nis")[:, :chk]
                use_idmm = cfg['npre'] == 'idmm'
                nc.tensor.matmul(rps, wt(l, 0, 0), x, start=True, stop=first)
                nc.tensor.matmul(zps, wt(l, 0, 1), x, start=True, stop=first)
                nc.tensor.matmul(nis, wt(l, 0, 2), x, start=True,
                                 stop=not use_idmm)
                if not first:
                    nhs = ps.tile([128, CH], F32, tag="nh", name="nhs")[:, :chk]
                    nc.tensor.matmul(rps, wt(l, 1, 0), h, start=False, stop=True)
                    nc.tensor.matmul(zps, wt(l, 1, 1), h, start=False, stop=True)
                    nc.tensor.matmul(nhs, wt(l, 1, 2), h, start=True, stop=True)
                r = small.tile([128, CH], FP16, name="r", tag="r", bufs=4)[:, :chk]
                z = small.tile([128, CH], FP16, name="z", tag="z", bufs=8)[:, :chk]
                nc.scalar.activation(r, rps, SIG, bias=bap(l, 0))
                nc.scalar.activation(z, zps, SIG, bias=bap(l, 1))
                tm = small.tile([128, CH], FP16, name="tm", tag="tm", bufs=6)[:, :chk]
                if first:
                    nc.vector.tensor_scalar_mul(tm, r, bap(l, 3))
                else:
                    nc.vector.scalar_tensor_tensor(tm, nhs, bap(l, 3), r,
                                                   op0=ADD, op1=MULT)
                pe_fill(fill)
                if use_idmm:
                    nc.tensor.matmul(nis, ident, tm, start=False, stop=True)
                    n_src, n_bias = nis, bap(l, 2)
                else:
                    npre = small.tile([128, CH], FP16, name="npre", tag="np",
                                      bufs=4)[:, :chk]
                    eng(cfg['npre']).scalar_tensor_tensor(
                        npre, nis, bap(l, 2), tm, op0=ADD, op1=ADD)
                    n_src, n_bias = npre, 0.0
                n = small.tile([128, CH], FP16, name="n", tag="n", bufs=6)[:, :chk]
                nc.scalar.activation(n, n_src, TANH, bias=n_bias)
                return (h, hp, z, n, first, cfg, chk)

            def emit_tail(state):
                h, hp, z, n, first, cfg, chk = state
                if first:
                    e = small.tile([128, CH], FP16, name="e", tag="d",
                                   bufs=4)[:, :chk]
                    nc.vector.tensor_mul(e, z, n)
                    nc.vector.tensor_sub(hp, n, e)
                else:
                    d = small.tile([128, CH], FP16, name="d", tag="d",
                                   bufs=4)[:, :chk]
                    eng(cfg['d']).tensor_sub(d, h, n)
                    u = small.tile([128, CH], FP16, name="u", tag="u",
                                   bufs=4)[:, :chk]
                    eng(cfg['u']).tensor_mul(u, z, d)
                    eng(cfg['hp']).tensor_add(hp, n, u)

            def emit_chunk(l, x, h, hp, first, cfg, fill=0):
                emit_tail(emit_head(l, x, h, hp, first, cfg, fill))

            ident = None
            if 'idmm' in (cfg0['npre'], cfg1['npre'], cfg2['npre']):
                from concourse.masks import make_identity
                ident = const.tile([H, H], FP16)
                make_identity(nc, ident)

            def emit_l0_heads(t):
                nch = CH // l0_streams
                states = []
                for s in range(l0_streams):
                    c0 = t * CH + s * nch
                    x = xt[:, 0, c0:c0 + nch]
                    h = x1[:, 0, c0 - CH:c0 - CH + nch] if t > 0 else None
                    hp = x1[:, 0, c0:c0 + nch]
                    states.append(emit_head(0, x, h, hp, t == 0, cfg0,
                                            fill=fill0))
                return states

            def emit_l1_chunk(s, k):
                c0 = s * 1024 + k * CH
                x = x1[:, 0, c0:c0 + CH]
                h = x2[:, 0, c0 - 1024:c0 - 1024 + CH] if s > 0 else None
                hp = x2[:, 0, c0:c0 + CH]
                emit_chunk(1, x, h, hp, s == 0, cfg1, fill=fill12)

            def emit_l1(s):
                for k in range(2):
                    emit_l1_chunk(s, k)

            stage_prev = [None, None]  # [previous step's tile, current]

            def emit_l2_chunk(p, k):
                # k==0 allocates step p's stage tile; k==3 appends the
                # output DMA and retires the step.
                if k == 0:
                    stage_prev[1] = stage_p.tile([128, 2048], FP16, tag="st",
                                                 name="st")
                st = stage_prev[1]
                c0 = p * 2048 + k * CH
                x = x2[:, 0, c0:c0 + CH]
                h = (stage_prev[0][:, k * CH:(k + 1) * CH]
                     if p > 0 else None)
                hp = st[:, k * CH:(k + 1) * CH]
                emit_chunk(2, x, h, hp, p == 0, cfg2, fill=fill12)
                if k == 3:
                    w = min(TOK - p * 2048, 2048)
                    nc.sync.dma_start(out=out_d[:, p * 2048:p * 2048 + w],
                                      in_=st[:, :w])
                    stage_prev[0] = st

            # Emission order note: each engine executes its stream IN ORDER,
            # so ready work must be emitted BEFORE stall-prone work. Per
            # window t we first emit the L1/L2 steps whose inputs completed
            # in earlier windows, then L0 step t (whose hh-matmuls stall on
            # h'(t-1)); the ready L1/L2 matmuls keep the PE busy through
            # L0's serial-chain hole.
            # Sandwich weave: [cover chunk, L0 streams, cover chunk] per
            # window. The leading cover chunk keeps engines busy while
            # L0(t)'s matmuls wait on h'(t-1); emitting L0 in the middle
            # puts its serial-chain ops near the front of every engine
            # queue, so h'(t) completes mid-window; the trailing cover
            # chunk refills the queues behind it. Even windows carry L1
            # chunks as cover, odd windows L2 chunks.
            def covers(t):
                if t % 2 == 0:
                    s = (t - 2) // 2
                    return [] if t < 2 else [('l1', s, 0), ('l1', s, 1)]
                if t % 4 == 1:
                    p = (t - 5) // 4
                    return [] if t < 5 else [('l2', p, 0), ('l2', p, 1)]
                p = (t - 7) // 4
                return [] if t < 7 else [('l2', p, 2), ('l2', p, 3)]

            def emit_cover(c):
                if c[0] == 'l1':
                    emit_l1_chunk(c[1], c[2])
                else:
                    emit_l2_chunk(c[1], c[2])

            for t in range(T):
                cs = covers(t)
                if cs:
                    emit_cover(cs[0])
                for st_ in emit_l0_heads(t):
                    emit_tail(st_)
                if cs:
                    emit_cover(cs[1])
            emit_l1(24)
            emit_l2_chunk(11, 2)
            emit_l2_chunk(11, 3)
            for k in range(4):
                emit_l2_chunk(12, k)

    nc.finalize()
    return nc


def _get_runner():
    if "runner" in _CACHE:
        return _CACHE["runner"]
    import jax
    import numpy as _np
    from jax.sharding import Mesh, PartitionSpec
    from jax.experimental.shard_map import shard_map
    import concourse.bass2jax as bass2jax
    import concourse.mybir as mybir

    nc = _CACHE.get("nc") or _build()
    _CACHE["nc"] = nc
    bass2jax.install_neuronx_cc_hook()

    partition_name = nc.partition_id_tensor.name if nc.partition_id_tensor else None
    in_names, out_names, out_avals, zero_outs = [], [], [], []
    for alloc in nc.m.functions[0].allocations:
        if not isinstance(alloc, mybir.MemoryLocationSet):
            continue
        name = alloc.memorylocations[0].name
        if alloc.kind == "ExternalInput":
            if name != partition_name:
                in_names.append(name)
        elif alloc.kind == "ExternalOutput":
            out_avals.append(jax.core.ShapedArray(
                tuple(alloc.tensor_shape), mybir.dt.np(alloc.dtype)))
            zero_outs.append(_np.zeros(alloc.tensor_shape, mybir.dt.np(alloc.dtype)))
            out_names.append(name)

    n_params = len(in_names)
    all_in_names = list(in_names) + list(out_names)
    if partition_name is not None:
        all_in_names.append(partition_name)

    donate = tuple(range(n_params, n_params + len(out_names)))

    def _body(*args):
        operands = list(args)
        if partition_name is not None:
            operands.append(bass2jax.partition_id_tensor())
        outs = bass2jax._bass_exec_p.bind(
            *operands,
            out_avals=tuple(out_avals),
            in_names=tuple(all_in_names),
            out_names=tuple(out_names),
            lowering_input_output_aliases=(),
            sim_require_finite=True,
            sim_require_nnan=True,
            nc=nc,
        )
        return tuple(outs)

    devices = jax.devices()[:NCORES]
    mesh = Mesh(_np.asarray(devices), ("core",))
    in_specs = (PartitionSpec("core"),) * (n_params + len(out_names))
    out_specs = (PartitionSpec("core"),) * len(out_names)
    sharded = jax.jit(
        shard_map(_body, mesh=mesh, in_specs=in_specs, out_specs=out_specs,
                  check_rep=False),
        donate_argnums=donate, keep_unused=True)

    def run(in_maps):
        concat_in = [
            _np.concatenate([_np.asarray(m[name]) for m in in_maps], axis=0)
            for name in in_names
        ]
        concat_zeros = [
            _np.zeros((NCORES * z.shape[0], *z.shape[1:]), z.dtype)
            for z in zero_outs
        ]
        out_arrs = sharded(*concat_in, *concat_zeros)
        o = _np.asarray(out_arrs[out_names.index("out")])
        return o.reshape(NCORES, 128, TOK)

    _CACHE["runner"] = run
    return run


def make_core_inputs(text_sh, emb16, w_np, b_np):
    """Per-core input dict: host embedding lookup in transposed t-major fp16."""
    x = emb16[text_sh]                                   # [BC, T, H] fp16
    xt = np.ascontiguousarray(x.transpose(2, 1, 0)).reshape(128, TOK)
    return {"xt": xt, "wts": w_np, "bias": b_np}


def pack_weights(params):
    w_np = np.zeros((128, LAYERS * 2 * 3 * H), np.float16)
    b_np = np.zeros((128, LAYERS * 4), np.float32)
    for l, (Wih, Whh, bih, bhh) in enumerate(params):
        for g in range(3):
            w_np[:, (l * 6 + g) * H:(l * 6 + g + 1) * H] = \
                Wih[g * H:(g + 1) * H, :].T.astype(np.float16)
            w_np[:, (l * 6 + 3 + g) * H:(l * 6 + 3 + g + 1) * H] = \
                Whh[g * H:(g + 1) * H, :].T.astype(np.float16)
        b_np[:, l * 4 + 0] = bih[0:H] + bhh[0:H]
        b_np[:, l * 4 + 1] = bih[H:2 * H] + bhh[H:2 * H]
        b_np[:, l * 4 + 2] = bih[2 * H:3 * H]
        b_np[:, l * 4 + 3] = bhh[2 * H:3 * H]
    return w_np, b_np


def prepare_in_maps(text_inputs, emb, params):
    emb16 = np.asarray(emb, np.float32).astype(np.float16)
    w_np, b_np = pack_weights(params)
    return [
        make_core_inputs(np.asarray(text_inputs)[c * BC:(c + 1) * BC],
                         emb16, w_np, b_np)
        for c in range(NCORES)
    ]


def finish_output(o, text_inputs):
    """[NCORES,128,TOK] fp16 -> [B,T,H] f32 with empty-sentence masking."""
    o = o.reshape(NCORES, 128, T, BC).transpose(0, 3, 2, 1)  # [c, b, t, h]
    out = np.ascontiguousarray(o).reshape(B, T, H).astype(np.float32)
    lens = np.sign(np.asarray(text_inputs)).sum(axis=1)
    out *= (lens > 0).astype(np.float32)[:, None, None]
    return out


def kernel(text_inputs, mask_input, len_seq, emb,
           Wih0, Whh0, bih0, bhh0, Wih1, Whh1, bih1, bhh1,
           Wih2, Whh2, bih2, bhh2):
    run = _get_runner()
    params = [(np.asarray(Wih0, np.float32), np.asarray(Whh0, np.float32),
               np.asarray(bih0, np.float32), np.asarray(bhh0, np.float32)),
              (np.asarray(Wih1, np.float32), np.asarray(Whh1, np.float32),
               np.asarray(bih1, np.float32), np.asarray(bhh1, np.float32)),
              (np.asarray(Wih2, np.float32), np.asarray(Whh2, np.float32),
               np.asarray(bih2, np.float32), np.asarray(bhh2, np.float32))]
    in_maps = prepare_in_maps(text_inputs, emb, params)
    o = run(in_maps)
    return finish_output(o, text_inputs)
